# revision 22
# baseline (speedup 1.0000x reference)
"""DendriNet Trainium2 kernel (v2: fp8 DoubleRow + fast exact top-k).

Computation (see reference): 3 branch layers, each doing
  h = (exc + cur) / (exc + 1 + sum_cond + inh_term)
with exc = x @ Wexc.T, inh_term = inh @ Winh.T, and W* = top32-masked exp(pre_w),
followed by a soma nonlinearity  rate = exp(la) * relu(v - sigmoid(vth))^2.

Numerics: matmuls run in fp8 e4m3 with DoubleRow (2x PE rate).  To keep the
error down, inputs are mean-centered (y = x - 0.5) before quantization and the
exact 0.5*rowsum(W) term is added back as a per-row bias in the combine stage.
This kills the dominant (weight-mean) quantization error: simulated l2 5.9e-3.

Launch 1 (prep, tensor-parallel over 672 weight rows/core):
  - exact per-row top-32 threshold via hierarchical pair-max/min halving:
    top32(w) is contained in top32(qmax) u top16(qmin) u top16(rmax) u top8(rmin)
    where z* = pairwise max/min of row halves and q*/r* = pairwise of z-halves.
    DVE max8/match_replace rounds then run on 1024-wide arrays (not 4096),
    and the level-1 builds + mask-select run on GpSimd in parallel.
  - W = (w >= t32) * exp(w) in one scalar_tensor_tensor with fused rowsum
    (accum_out); transposed via PE matmul-against-identity; stored fp8.
  - x/inh shards: cast to bf16 with -0.5 bias on ACT, PE-transposed, fp8.
Launch 2 (main, data-parallel over batch, 512 rows/core):
  - 21 output groups of 128 rows; per group 2x16 DoubleRow fp8 matmuls
    (256-deep contraction each) into rolling PSUM banks.
  - combine: esb = psum_e + 0.5*rsE (ACT Identity w/ bias), den = esb + cvec
    + psum_i (DVE stt), rec = reciprocal_approx_fast, h = esb * rec.
  - branch-tree aggregation via small block-diagonal bf16 matmuls as before.
"""

import os
import sys

for _p in ("/opt/trn_rl_repo",):
    if os.path.isdir(_p) and _p not in sys.path:
        sys.path.insert(0, _p)

import numpy as np
import ml_dtypes

import concourse.bass as bass
import concourse.tile as tile
from concourse import bacc, mybir
from concourse.bass_utils import run_bass_kernel_spmd
from concourse.masks import make_identity

BF16 = ml_dtypes.bfloat16
E4M3 = ml_dtypes.float8_e4m3
F32 = np.float32

NCORES = 8
B = 4096
D = 4096
BS = B // NCORES          # 512 batch rows per core
K = 32                    # top-k per weight row

O0, O1, OS = 2048, 512, 128
PC0, PC1, PCS = O0 // NCORES, O1 // NCORES, OS // NCORES   # 256, 64, 16
ROWS_PC = 2 * (PC0 + PC1 + PCS)                            # 672
ROWS_PAD = 768                                             # 6 tiles of 128
NT = ROWS_PAD // 128                                       # 6 weight tiles
NG = (O0 + O1 + OS) // 128                                 # 21 output groups

FP_MIN = -1e30
DT = mybir.dt
AF = mybir.ActivationFunctionType
OP = mybir.AluOpType

LAST_PROFILE = {}


def _new_nc():
    return bacc.Bacc(
        "TRN2", target_bir_lowering=False, debug=False, num_devices=NCORES)


# ----------------------------------------------------------------- launch 1

def build_prep_kernel():
    nc = _new_nc()
    prew = nc.dram_tensor("prew", [ROWS_PAD, D], DT.float32, kind="ExternalInput")
    xs = nc.dram_tensor("xs", [BS, D], DT.float32, kind="ExternalInput")
    ins = nc.dram_tensor("ins", [BS, D], DT.float32, kind="ExternalInput")
    wtc = nc.dram_tensor("wtc", [NT, 128, 32, 128], DT.float8e4, kind="ExternalOutput")
    rs2 = nc.dram_tensor("rs2", [128, NT], DT.float32, kind="ExternalOutput")
    yt = nc.dram_tensor("yt", [4, 128, 32, 128], DT.float8e4, kind="ExternalOutput")
    itb = nc.dram_tensor("itb", [4, 128, 32, 128], DT.float8e4, kind="ExternalOutput")

    with tile.TileContext(nc) as tc:
        with (
            tc.tile_pool(name="consts", bufs=1) as consts,
            tc.tile_pool(name="rowp", bufs=2) as rowp,
            tc.tile_pool(name="zp", bufs=2) as zp,
            tc.tile_pool(name="qp", bufs=1) as qp,
            tc.tile_pool(name="candp", bufs=2) as candp,
            tc.tile_pool(name="expp", bufs=2) as expp,
            tc.tile_pool(name="wbp", bufs=2) as wbp,
            tc.tile_pool(name="rsp", bufs=1) as rsp,
            tc.tile_pool(name="stg", bufs=2) as stg,
            tc.tile_pool(name="xrow", bufs=2) as xrow,
            tc.tile_pool(name="ybp", bufs=2) as ybp,
            tc.tile_pool(name="tp", bufs=4, space="PSUM") as tp,
        ):
            id_bf = consts.tile([128, 128], DT.bfloat16)
            make_identity(nc, id_bf)
            nhalf = consts.tile([128, 1], DT.float32)
            nc.gpsimd.memset(nhalf, -0.5)
            rs_sb = rsp.tile([128, NT], DT.float32)

            def transpose_out(src_bf, dst_dram, stage_tag):
                """PE-transpose [128, 4096] -> staged fp8 -> one DMA."""
                st = stg.tile([128, 32, 128], DT.float8e4, tag=stage_tag)
                for jj in range(8):
                    pt = tp.tile([128, 4, 128], DT.float32, tag="pt")
                    for m in range(4):
                        j = 4 * jj + m
                        nc.tensor.matmul(
                            pt[:, m, :], src_bf[:, 128 * j:128 * (j + 1)],
                            id_bf, start=True, stop=True)
                    nc.any.tensor_copy(st[:, 4 * jj:4 * jj + 4, :], pt)
                nc.sync.dma_start(out=dst_dram, in_=st)

            xjobs = [(src, dst, bt)
                     for src, dst in ((xs, yt), (ins, itb))
                     for bt in range(4)]

            def emit_xjob(src, dst, bt):
                xl = xrow.tile([128, D], DT.float32, tag="xl")
                nc.sync.dma_start(
                    out=xl[:, :2048], in_=src[128 * bt:128 * (bt + 1), :2048])
                nc.sync.dma_start(
                    out=xl[:, 2048:], in_=src[128 * bt:128 * (bt + 1), 2048:])
                y8 = ybp.tile([128, D], DT.float8e4, tag="y8")
                # y = x - 0.5 (mean-center before fp8; single rounding)
                nc.scalar.activation(y8, xl, AF.Identity, bias=nhalf)
                transpose_out(y8, dst[bt], "xst")

            xj = 0
            for t in range(NT):
                orig = rowp.tile([128, D], DT.float32, tag="orig")
                nc.sync.dma_start(
                    out=orig[:, :2048], in_=prew[128 * t:128 * (t + 1), :2048])
                nc.sync.dma_start(
                    out=orig[:, 2048:], in_=prew[128 * t:128 * (t + 1), 2048:])
                exh = expp.tile([128, D], DT.bfloat16, tag="exh")
                nc.scalar.activation(exh, orig, AF.Exp)

                # level-1 pair max/min of row halves on Pool+ACT:
                #   max(a,b) = a + relu(b-a),  min(a,b) = b - relu(b-a)
                lo, hi = orig[:, :2048], orig[:, 2048:]
                d1 = zp.tile([128, 2048], DT.float32, tag="d1")
                nc.gpsimd.tensor_sub(d1, hi, lo)
                nc.scalar.activation(d1, d1, AF.Relu)
                zmx = zp.tile([128, 2048], DT.float32, tag="zmx")
                nc.gpsimd.tensor_add(zmx, lo, d1)
                zmn = zp.tile([128, 2048], DT.float32, tag="zmn")
                nc.gpsimd.tensor_sub(zmn, hi, d1)
                # level-2 pairs all on DVE (keeps the rounds section
                # free of cross-engine waits)
                qmx = qp.tile([128, 1024], DT.float32, tag="qmx")
                nc.vector.tensor_tensor(qmx, zmx[:, :1024], zmx[:, 1024:], OP.max)
                qmn = qp.tile([128, 1024], DT.float32, tag="qmn")
                nc.vector.tensor_tensor(qmn, zmx[:, :1024], zmx[:, 1024:], OP.min)
                rmx = qp.tile([128, 1024], DT.float32, tag="rmx")
                nc.vector.tensor_tensor(rmx, zmn[:, :1024], zmn[:, 1024:], OP.max)
                rmn = qp.tile([128, 1024], DT.float32, tag="rmn")
                nc.vector.tensor_tensor(rmn, zmn[:, :1024], zmn[:, 1024:], OP.min)

                cand = candp.tile([128, 72], DT.float32, tag="cand")

                def rounds(arr, n_rounds, base):
                    for r in range(n_rounds):
                        nc.vector.max(cand[:, base + 8 * r:base + 8 * (r + 1)], arr)
                        if r != n_rounds - 1:
                            nc.vector.match_replace(
                                arr, cand[:, base + 8 * r:base + 8 * (r + 1)],
                                arr, FP_MIN)

                rounds(qmx, 4, 0)    # top-32 of qmx
                rounds(qmn, 2, 32)   # top-16 of qmn
                rounds(rmx, 2, 48)   # top-16 of rmx
                rounds(rmn, 1, 64)   # top-8  of rmn

                # merge candidates: rank-32 of the 72 values
                mfin = None
                for r in range(4):
                    m8 = candp.tile([128, 8], DT.float32, tag=f"m{r}")
                    nc.vector.max(m8, cand)
                    if r != 3:
                        nc.vector.match_replace(cand, m8, cand, FP_MIN)
                    mfin = m8
                t32 = mfin[:, 7:8]

                # W = (w >= t32) * exp(w), fused rowsum (one DVE pass)
                wb = wbp.tile([128, D], DT.bfloat16, tag="wb")
                nc.vector.scalar_tensor_tensor(
                    out=wb, in0=orig, scalar=t32, in1=exh,
                    op0=OP.is_ge, op1=OP.mult,
                    accum_out=rs_sb[:, t:t + 1])

                transpose_out(wb, wtc[t], "wst")

                # spread the 8 x/inh jobs over the 6 W tiles
                want = (t + 1) * len(xjobs) // NT
                while xj < want:
                    emit_xjob(*xjobs[xj])
                    xj += 1

            nc.sync.dma_start(out=rs2[:, :], in_=rs_sb)
    nc.compile()
    return nc


# ----------------------------------------------------------------- launch 2

def build_main_kernel():
    nc = _new_nc()
    wt2 = nc.dram_tensor("wt2", [2 * NG, 128, 16, 2, 128], DT.float8e4,
                         kind="ExternalInput")
    xt = nc.dram_tensor("xt", [4, 128, 32, 128], DT.float8e4, kind="ExternalInput")
    it = nc.dram_tensor("it", [4, 128, 32, 128], DT.float8e4, kind="ExternalInput")
    s1 = nc.dram_tensor("s1", [16, 128, 128], DT.bfloat16, kind="ExternalInput")
    ss = nc.dram_tensor("ss", [4, 128, 128], DT.bfloat16, kind="ExternalInput")
    cvb = nc.dram_tensor("cvb", [128, NG], DT.float32, kind="ExternalInput")
    beb = nc.dram_tensor("beb", [128, NG], DT.float32, kind="ExternalInput")
    vth = nc.dram_tensor("vth", [128, 1], DT.float32, kind="ExternalInput")
    alp = nc.dram_tensor("alp", [128, 1], DT.float32, kind="ExternalInput")
    out = nc.dram_tensor("rate", [OS, BS], DT.float32, kind="ExternalOutput")

    DR = mybir.MatmulPerfMode.DoubleRow

    with tile.TileContext(nc) as tc:
        with (
            tc.tile_pool(name="res", bufs=1) as res,
            tc.tile_pool(name="wch", bufs=4) as wch,
            tc.tile_pool(name="h0p", bufs=1) as h0p,
            tc.tile_pool(name="h1p", bufs=1) as h1p,
            tc.tile_pool(name="cmb", bufs=2) as cmb,
            tc.tile_pool(name="mm", bufs=1, space="PSUM") as mm,
        ):
            # xt first (in 16 slices for queue parallelism), then the first
            # weight chunk, so the first matmul can start ~10us in; it_sb
            # and the small tables stream in behind.
            xt_sb = res.tile([128, 32, 512], DT.float8e4, name="xt_sb")
            it_sb = res.tile([128, 32, 512], DT.float8e4, name="it_sb")
            for bt in range(4):
                for ah in range(4):
                    nc.sync.dma_start(
                        out=xt_sb[:, 8 * ah:8 * (ah + 1),
                                  128 * bt:128 * (bt + 1)],
                        in_=xt[bt][:, 8 * ah:8 * (ah + 1), :])

            def load_chunk(gi, tag):
                ch = wch.tile([128, 16, 2, 128], DT.float8e4, tag=tag)
                nc.sync.dma_start(out=ch[:, :8], in_=wt2[gi][:, :8])
                nc.sync.dma_start(out=ch[:, 8:], in_=wt2[gi][:, 8:])
                return ch

            che0 = load_chunk(0, "che")

            for bt in range(4):
                for ah in range(4):
                    nc.sync.dma_start(
                        out=it_sb[:, 8 * ah:8 * (ah + 1),
                                  128 * bt:128 * (bt + 1)],
                        in_=it[bt][:, 8 * ah:8 * (ah + 1), :])
            s1_sb = res.tile([128, 16, 128], DT.bfloat16, name="s1_sb")
            nc.sync.dma_start(out=s1_sb, in_=s1.rearrange("k p c -> p k c"))
            ss_sb = res.tile([128, 4, 128], DT.bfloat16, name="ss_sb")
            nc.sync.dma_start(out=ss_sb, in_=ss.rearrange("k p c -> p k c"))
            cv_sb = res.tile([128, NG], DT.float32, name="cv_sb")
            nc.sync.dma_start(out=cv_sb, in_=cvb[:, :])
            be_sb = res.tile([128, NG], DT.float32, name="be_sb")
            nc.sync.dma_start(out=be_sb, in_=beb[:, :])
            vth_sb = res.tile([128, 1], DT.float32, name="vth_sb")
            nc.sync.dma_start(out=vth_sb, in_=vth[:, :])
            al_sb = res.tile([128, 1], DT.float32, name="al_sb")
            nc.sync.dma_start(out=al_sb, in_=alp[:, :])

            h0t = [h0p.tile([128, 512], DT.bfloat16, tag=f"h0_{k}", name=f"h0_{k}")
                   for k in range(16)]
            h1t = [h1p.tile([128, 512], DT.bfloat16, tag=f"h1_{k}", name=f"h1_{k}")
                   for k in range(4)]

            for g in range(NG):
                che = che0 if g == 0 else load_chunk(2 * g, "che")
                chi = load_chunk(2 * g + 1, "chi")
                pse = mm.tile([128, 512], DT.float32, tag=f"e{g % 3}",
                              name=f"pse{g}")
                psi = mm.tile([128, 512], DT.float32, tag=f"i{g % 3}",
                              name=f"psi{g}")
                for d2 in range(16):
                    nc.tensor.matmul(
                        pse, che[:, d2], xt_sb[:, 2 * d2:2 * d2 + 2, :],
                        start=(d2 == 0), stop=(d2 == 15), perf_mode=DR)
                for d2 in range(16):
                    nc.tensor.matmul(
                        psi, chi[:, d2], it_sb[:, 2 * d2:2 * d2 + 2, :],
                        start=(d2 == 0), stop=(d2 == 15), perf_mode=DR)

                esb = cmb.tile([128, 512], DT.float32, tag="esb")
                nc.scalar.activation(esb, pse, AF.Identity,
                                     bias=be_sb[:, g:g + 1])
                den = cmb.tile([128, 512], DT.float32, tag="den")
                nc.vector.scalar_tensor_tensor(
                    out=den, in0=esb, scalar=cv_sb[:, g:g + 1], in1=psi,
                    op0=OP.add, op1=OP.add)
                rec = cmb.tile([128, 512], DT.float32, tag="rec")
                nc.vector.reciprocal_approx_fast(rec, den)

                if g < 16:
                    nc.vector.tensor_mul(h0t[g], esb, rec)
                elif g < 20:
                    ot = g - 16
                    cur = mm.tile([128, 512], DT.float32, tag="cur",
                                  name=f"cur{g}")
                    for m in range(4):
                        kk = 4 * ot + m
                        nc.tensor.matmul(cur, s1_sb[:, kk, :], h0t[kk],
                                         start=(m == 0), stop=(m == 3))
                    num = cmb.tile([128, 512], DT.float32, tag="num")
                    nc.vector.tensor_add(num, esb, cur)
                    nc.vector.tensor_mul(h1t[ot], num, rec)
                else:
                    cur = mm.tile([128, 512], DT.float32, tag="cur",
                                  name=f"cur{g}")
                    for m in range(4):
                        nc.tensor.matmul(cur, ss_sb[:, m, :], h1t[m],
                                         start=(m == 0), stop=(m == 3))
                    num = cmb.tile([128, 512], DT.float32, tag="num")
                    nc.vector.tensor_add(num, esb, cur)
                    v = cmb.tile([128, 512], DT.float32, tag="v")
                    nc.vector.tensor_mul(v, num, rec)
                    vd = cmb.tile([128, 512], DT.float32, tag="vd")
                    nc.vector.tensor_scalar(
                        vd, v, vth_sb, None, op0=OP.subtract)
                    rr = cmb.tile([128, 512], DT.float32, tag="rr")
                    nc.scalar.activation(rr, vd, AF.Relu)
                    rt = cmb.tile([128, 512], DT.float32, tag="rt")
                    nc.vector.scalar_tensor_tensor(
                        out=rt, in0=rr, scalar=al_sb, in1=rr,
                        op0=OP.mult, op1=OP.mult)
                    nc.sync.dma_start(out=out[:, :], in_=rt)
    nc.compile()
    return nc


# ----------------------------------------------------------------- host glue

def _build_s_mats(block_w1, block_w_s):
    bw1f = np.asarray(block_w1, F32).reshape(-1)       # [2048]
    bwsf = np.asarray(block_w_s, F32).reshape(-1)      # [512]
    p = np.arange(128)
    s1 = np.zeros((16, 128, 128), F32)
    for k in range(16):
        c = 32 * (k % 4) + p // 4
        s1[k, p, c] = bw1f[128 * k + p]
    ssm = np.zeros((4, 128, 128), F32)
    for m in range(4):
        c = 32 * m + p // 4
        ssm[m, p, c] = bwsf[128 * m + p]
    return s1.astype(BF16), ssm.astype(BF16)


_CACHE = {}


class _ldw_opt:
    """Swap --enable-ldw-opt=false -> true so FWL (fast weight load) kicks in.
    Scoped: walrus rejects ldw-opt on DoubleRow Ldweights, so only the prep
    kernel (plain bf16 transposes) compiles with it."""

    def __enter__(self):
        import concourse.bass_utils as bu
        self.bu = bu
        self.orig = bu.run_command

        def patched(cmd, **kw):
            cmd = ["--enable-ldw-opt=true" if c == "--enable-ldw-opt=false"
                   else c for c in cmd]
            return self.orig(cmd, **kw)

        bu.run_command = patched
        return self

    def __exit__(self, *a):
        self.bu.run_command = self.orig
        return False


def _install_ntff_hook():
    """bass_utils' trace path looks up antenv.axon_hooks, which this image
    lacks; synthesize it and register the ctypes NTFF hook."""
    import types
    if "antenv.axon_hooks" in sys.modules:
        return
    try:
        from trn_agent_boot.trn_boot import _ntff_profile_via_ctypes
        hook = _ntff_profile_via_ctypes("/opt/axon/libaxon_pjrt.so")
    except Exception:
        hook = None
    mod = types.ModuleType("antenv.axon_hooks")
    _h = [hook]
    mod.set_axon_ntff_profile_hook = lambda h: _h.__setitem__(0, h)
    mod.get_axon_ntff_profile_hook = lambda: _h[0]
    sys.modules["antenv.axon_hooks"] = mod
    try:
        import antenv
        antenv.axon_hooks = mod
    except Exception:
        pass


def _chunk(subT):
    """[4096 d, 128 c] fp8 -> [128 p, 16 d2, 2 j, 128 c] (d = 128*(2*d2+j)+p)."""
    return np.ascontiguousarray(
        subT.reshape(16, 2, 128, 128).transpose(2, 0, 1, 3))


def kernel(x, inhibitory_input, pre_w_exc0, pre_w_inh0, pre_w_exc1, pre_w_inh1,
           block_w1, pre_w_exc_s, pre_w_inh_s, block_w_s, presigmoid_Vth,
           log_alpha_max):
    x = np.ascontiguousarray(np.asarray(x, F32))
    inh = np.ascontiguousarray(np.asarray(inhibitory_input, F32))
    e0 = np.asarray(pre_w_exc0, F32)
    i0 = np.asarray(pre_w_inh0, F32)
    e1 = np.asarray(pre_w_exc1, F32)
    i1 = np.asarray(pre_w_inh1, F32)
    es = np.asarray(pre_w_exc_s, F32)
    is_ = np.asarray(pre_w_inh_s, F32)

    if "prep" not in _CACHE:
        _CACHE["prep"] = build_prep_kernel()
        _CACHE["main"] = build_main_kernel()
    trace = bool(os.environ.get("BASS_TRACE"))
    if trace:
        _install_ntff_hook()

    in_maps = []
    for c in range(NCORES):
        prew = np.concatenate([
            e0[PC0 * c:PC0 * (c + 1)], e1[PC1 * c:PC1 * (c + 1)],
            es[PCS * c:PCS * (c + 1)],
            i0[PC0 * c:PC0 * (c + 1)], i1[PC1 * c:PC1 * (c + 1)],
            is_[PCS * c:PCS * (c + 1)],
            np.zeros((ROWS_PAD - ROWS_PC, D), F32),
        ])
        in_maps.append({
            "prew": np.ascontiguousarray(prew),
            "xs": x[BS * c:BS * (c + 1)],
            "ins": inh[BS * c:BS * (c + 1)],
        })
    r1 = run_bass_kernel_spmd(
        _CACHE["prep"], in_maps, core_ids=list(range(NCORES)), trace=trace)
    LAST_PROFILE["prep_ns"] = r1.exec_time_ns

    # ---- reassemble per-table W.T (fp8) and rowsums (f32)
    # local col layout per core: e0[0:256] e1[256:320] es[320:336]
    #                            i0[336:592] i1[592:656] is[656:672]
    e0T = np.empty((D, O0), E4M3)
    i0T = np.empty((D, O0), E4M3)
    e1T = np.empty((D, O1), E4M3)
    i1T = np.empty((D, O1), E4M3)
    esT = np.empty((D, OS), E4M3)
    isT = np.empty((D, OS), E4M3)
    rsE = np.empty(O0 + O1 + OS, F32)
    rsI = np.empty(O0 + O1 + OS, F32)
    for c in range(NCORES):
        wtc = np.asarray(r1.results[c]["wtc"])          # [6,128,32,128] fp8
        WlT = wtc.transpose(2, 1, 0, 3).reshape(D, ROWS_PAD)
        rs2 = np.asarray(r1.results[c]["rs2"], F32)     # [128, 6]
        rsl = rs2.T.reshape(ROWS_PAD)
        e0T[:, PC0 * c:PC0 * (c + 1)] = WlT[:, 0:256]
        e1T[:, PC1 * c:PC1 * (c + 1)] = WlT[:, 256:320]
        esT[:, PCS * c:PCS * (c + 1)] = WlT[:, 320:336]
        i0T[:, PC0 * c:PC0 * (c + 1)] = WlT[:, 336:592]
        i1T[:, PC1 * c:PC1 * (c + 1)] = WlT[:, 592:656]
        isT[:, PCS * c:PCS * (c + 1)] = WlT[:, 656:672]
        rsE[PC0 * c:PC0 * (c + 1)] = rsl[0:256]
        rsE[O0 + PC1 * c:O0 + PC1 * (c + 1)] = rsl[256:320]
        rsE[O0 + O1 + PCS * c:O0 + O1 + PCS * (c + 1)] = rsl[320:336]
        rsI[PC0 * c:PC0 * (c + 1)] = rsl[336:592]
        rsI[O0 + PC1 * c:O0 + PC1 * (c + 1)] = rsl[592:656]
        rsI[O0 + O1 + PCS * c:O0 + O1 + PCS * (c + 1)] = rsl[656:672]

    wt2 = np.empty((2 * NG, 128, 16, 2, 128), E4M3)
    for g in range(16):
        wt2[2 * g] = _chunk(e0T[:, 128 * g:128 * (g + 1)])
        wt2[2 * g + 1] = _chunk(i0T[:, 128 * g:128 * (g + 1)])
    for ot in range(4):
        g = 16 + ot
        wt2[2 * g] = _chunk(e1T[:, 128 * ot:128 * (ot + 1)])
        wt2[2 * g + 1] = _chunk(i1T[:, 128 * ot:128 * (ot + 1)])
    wt2[2 * 20] = _chunk(esT)
    wt2[2 * 20 + 1] = _chunk(isT)

    bw1 = np.asarray(block_w1, F32).reshape(O1, 4)
    bws = np.asarray(block_w_s, F32).reshape(OS, 4)
    sc = np.concatenate([np.zeros(O0, F32), bw1.sum(1), bws.sum(1)])
    beb = np.ascontiguousarray((0.5 * rsE).reshape(NG, 128).T.astype(F32))
    cvb = np.ascontiguousarray(
        (1.0 + sc + 0.5 * rsI).reshape(NG, 128).T.astype(F32))
    vthv = (1.0 / (1.0 + np.exp(-np.asarray(presigmoid_Vth, F32)))) \
        .reshape(OS, 1).astype(F32)
    alpv = np.exp(np.asarray(log_alpha_max, F32)).reshape(OS, 1).astype(F32)
    s1m, ssm = _build_s_mats(block_w1, block_w_s)

    in_maps2 = []
    for c in range(NCORES):
        in_maps2.append({
            "wt2": wt2,
            "xt": np.ascontiguousarray(np.asarray(r1.results[c]["yt"])),
            "it": np.ascontiguousarray(np.asarray(r1.results[c]["itb"])),
            "s1": s1m, "ss": ssm, "cvb": cvb, "beb": beb,
            "vth": vthv, "alp": alpv,
        })
    r2 = run_bass_kernel_spmd(
        _CACHE["main"], in_maps2, core_ids=list(range(NCORES)), trace=trace)
    LAST_PROFILE["main_ns"] = r2.exec_time_ns

    outp = np.empty((B, OS), F32)
    for c in range(NCORES):
        outp[BS * c:BS * (c + 1), :] = np.asarray(r2.results[c]["rate"], F32).T
    return outp
